# Initial kernel scaffold
#
"""Trainium2 Bass kernel for nn_CustomPenaltyLayer (MinMax-inverse penalty loss).

Contract: kernel(**inputs) takes the FULL inputs (x:(1024,4096,8) f32,
min_:(8,), scale_:(8,)) and returns the FULL output (scalar f32), sharding
x row-wise across 8 NeuronCores internally.

Math (reference):
  x_inv = (x.reshape(-1, 8) - min_) / scale_
  d = x_inv[:, 2]; a = x_inv[:, 3]
  dev_pen   = count(~(0 <= d <= 252))
  act_pen   = count(a < 0 or a > 22)
  trans_pen = sum over adjacent pairs of [mod(prev,2)==0 & prev<20] *
              [(cur != prev+1) & (cur != 22)]
  num_act   = count(a != 22);  total = dev+act+trans + |num_act - 58|

Device strategy (per core, data-parallel rows). The kernel is DMA-bound
(~53 us to stream 16 MiB/core at ~340 GB/s), so compute is balanced
across ScalarE and VectorE to stay under the per-tile DMA time, and the
tile schedule tapers at the end so the last tile's compute tail is short.
  - DMA: x in [128, R_t, 8] f32 tiles, R_t = [1024,1024,1024,512,256,256].
  - ScalarE: a3 = (v3-min3)*rs3, a2 = (v2-min2)*rs2 (strided reads), and
    the two a3 range counts as sign-sums via activation accum_out
    (count(a<t) = (T - sum sign(a-t))/2 up to measure-zero boundary hits).
  - VectorE: count(a3 != 22) (shifted bf16 predicate output, so the
    transition product-sums hit the DVE 2x bf16 mode), the a2 range pair
    via clamp (count(clamp(a2,0,252) != a2)), and the transition term via
    the identity pen = cond*ne22s - cond*eq1 (the 3-factor product
    cond*(1-eq1)*ne22s reduces to it because cond & eq1 & (cur==22)
    requires prev==21, which is odd and fails cond). "a3 is an even
    integer" uses the 2^23 magic-number round trick (no mod ALU op).
  - Pairs spanning the R_t-row partition chunks are computed on the host;
    partial sums are combined on the host into the final scalar.
"""

import os
import sys

for _p in ("/opt/trn_rl_repo", os.path.expanduser("~/.axon_site/_ro/trn_rl_repo")):
    if os.path.isdir(_p) and _p not in sys.path:
        sys.path.append(_p)

import numpy as np

import concourse.bacc as bacc
import concourse.tile as tile
from concourse import mybir
from concourse.bass_utils import run_bass_kernel_spmd

F32 = mybir.dt.float32
BF16 = mybir.dt.bfloat16
ALU = mybir.AluOpType
ACTF = mybir.ActivationFunctionType

MAGIC = 8388608.0  # 2^23
BATCH, TIMESTEPS, D = 1024, 4096, 8
N_ROWS = BATCH * TIMESTEPS          # 4,194,304
N_CORES = 8
ROWS_PER_CORE = N_ROWS // N_CORES   # 524,288
P = 128                             # SBUF partitions
R_LIST = (256, 768, 1024, 1024, 512, 384, 128)   # rows/partition per tile
assert sum(R_LIST) * P == ROWS_PER_CORE
N_T = len(R_LIST)

_NC_CACHE = {}


def _build_nc(x_bufs: int = 3, work_bufs: int = 2):
    n_t = N_T
    nc = bacc.Bacc("TRN2", target_bir_lowering=False, debug=False)

    xs = nc.dram_tensor("xs", [ROWS_PER_CORE, 8], F32, kind="ExternalInput")
    consts = nc.dram_tensor("consts", [P, 8], F32, kind="ExternalInput")
    accS_d = nc.dram_tensor("accS", [P, 3 * n_t], F32, kind="ExternalOutput")
    accD_d = nc.dram_tensor("accD", [P, 3 * n_t], F32, kind="ExternalOutput")

    xs_flat = xs.ap()
    r_max = max(R_LIST)

    with tile.TileContext(nc) as tc:
        with (
            tc.tile_pool(name="xp", bufs=x_bufs) as xp,
            tc.tile_pool(name="ap_", bufs=3) as ap_pool,
            tc.tile_pool(name="wp", bufs=work_bufs) as wp,
            tc.tile_pool(name="acc", bufs=1) as accp,
        ):
            consts_sb = accp.tile([P, 8], F32, tag="consts")
            nc.sync.dma_start(consts_sb[:], consts.ap())
            # Absorb the consts-DMA wait into one dummy ACT op: the HW
            # Activation encoding has a single sync-wait slot, and the
            # loop's first ACT op must wait on the x-tile DMA instead.
            dummy = accp.tile([P, 1], F32, tag="dummy")
            nc.scalar.copy(dummy[:], consts_sb[:, 0:1])
            rs3 = consts_sb[:, 0:1]    # f32(1/scale3)
            b3 = consts_sb[:, 1:2]     # -min3*rs3
            rs2 = consts_sb[:, 2:3]    # f32(1/scale2)
            b2 = consts_sb[:, 3:4]     # -min2*rs2
            zero = consts_sb[:, 4:5]   # 0.0    (sign bias: a3 < 0 test)
            n22 = consts_sb[:, 5:6]    # -22.0  (sign bias: a3 > 22 test)

            accS = accp.tile([P, 3 * n_t], F32, tag="accS")   # ScalarE-owned
            accD = accp.tile([P, 3 * n_t], F32, tag="accD")   # VectorE-owned
            sgn = accp.tile([P, r_max], F32, tag="sgn")  # sign scratch

            off = 0
            for t, r in enumerate(R_LIST):
                x_t = xp.tile([P, r, 8], F32, tag="x")
                src = xs_flat[off:off + P * r, :].rearrange(
                    "(p r) d -> p r d", r=r)
                if r >= 512:  # split large transfers: slightly better overlap
                    h = r // 2
                    nc.sync.dma_start(x_t[:, :h, :], src[:, :h, :])
                    nc.sync.dma_start(x_t[:, h:, :], src[:, h:, :])
                else:
                    nc.sync.dma_start(x_t[:], src)
                off += P * r
                v2 = x_t[:, :, 2]
                v3 = x_t[:, :, 3]

                # ScalarE: affine transforms, 2 sign-counts on a3, and
                # ne22 = Square(sign(a3-22)) whose accum is count(a3 != 22).
                a3 = ap_pool.tile([P, r], F32, tag="a3")
                nc.scalar.activation(a3[:], v3, ACTF.Identity, bias=b3, scale=rs3)
                s22 = ap_pool.tile([P, r], F32, tag="s22")
                nc.scalar.activation(s22[:], a3[:], ACTF.Sign, bias=n22,
                                     accum_out=accS[:, 3 * t + 1:3 * t + 2])
                ne22 = ap_pool.tile([P, r], F32, tag="ne22")
                nc.scalar.activation(ne22[:], s22[:], ACTF.Square,
                                     accum_out=accS[:, 3 * t + 2:3 * t + 3])
                a2 = ap_pool.tile([P, r], F32, tag="a2")
                nc.scalar.activation(a2[:], v2, ACTF.Identity, bias=b2, scale=rs2)
                nc.scalar.activation(sgn[:, :r], a3[:], ACTF.Sign, bias=zero,
                                     accum_out=accS[:, 3 * t + 0:3 * t + 1])

                # VectorE. accD cols per tile:
                #   0: count(a2 out of [0,252])   1: s1   2: s2
                c2 = accD[:, 3 * t + 0:3 * t + 1]
                c3 = accD[:, 3 * t + 1:3 * t + 2]
                c4 = accD[:, 3 * t + 2:3 * t + 3]

                h2 = wp.tile([P, r], F32, tag="h2")
                nc.vector.tensor_scalar(h2[:], a3[:], 0.5, MAGIC,
                                        ALU.mult, ALU.add)
                r2 = wp.tile([P, r], F32, tag="r2")
                nc.vector.tensor_scalar(r2[:], h2[:], MAGIC, 2.0,
                                        ALU.subtract, ALU.mult)
                meq = wp.tile([P, r], F32, tag="meq")
                nc.vector.tensor_tensor(meq[:], r2[:], a3[:], ALU.is_equal)
                cond = wp.tile([P, r], BF16, tag="cond")
                nc.vector.scalar_tensor_tensor(cond[:], a3[:], 20.0, meq[:],
                                               ALU.is_lt, ALU.mult)
                eq1 = wp.tile([P, r], BF16, tag="eq1")
                nc.vector.scalar_tensor_tensor(eq1[:, :r - 1], a3[:, :r - 1], 1.0,
                                               a3[:, 1:r], ALU.add, ALU.is_equal)
                junk = wp.tile([P, r], BF16, tag="junk")
                # s1 = sum(cond[:-1] * ne22[1:])
                nc.vector.scalar_tensor_tensor(junk[:, :r - 1], cond[:, :r - 1],
                                               0.0, ne22[:, 1:r],
                                               ALU.add, ALU.mult, accum_out=c3)
                # s2 = sum(eq1 * cond[:-1])
                nc.vector.scalar_tensor_tensor(junk[:, :r - 1], eq1[:, :r - 1],
                                               0.0, cond[:, :r - 1],
                                               ALU.add, ALU.mult, accum_out=c4)
                # dev: count(clamp(a2, 0, 252) != a2)
                cl2 = wp.tile([P, r], F32, tag="cl2")
                nc.vector.tensor_scalar(cl2[:], a2[:], 0.0, 252.0,
                                        ALU.max, ALU.min)
                jf = wp.tile([P, r], F32, tag="jf")
                nc.vector.scalar_tensor_tensor(jf[:], cl2[:], 0.0, a2[:],
                                               ALU.add, ALU.not_equal,
                                               accum_out=c2)

            nc.sync.dma_start(accS_d.ap(), accS[:])
            nc.sync.dma_start(accD_d.ap(), accD[:])

    nc.compile()
    return nc


def _make_consts(min_, scale_):
    m = np.asarray(min_, dtype=np.float64)
    s = np.asarray(scale_, dtype=np.float64)
    rs3 = np.float32(1.0) / np.float32(s[3])
    rs2 = np.float32(1.0) / np.float32(s[2])
    vals = np.array([
        np.float64(rs3),
        -np.float64(np.float32(m[3])) * np.float64(rs3),
        np.float64(rs2),
        -np.float64(np.float32(m[2])) * np.float64(rs2),
        0.0,
        -22.0,
        0.0,
        0.0,
    ], dtype=np.float64).astype(np.float32)
    return np.broadcast_to(vals, (P, 8)).copy()


def _run_device(x_flat, min_, scale_, trace=False):
    if "nc" not in _NC_CACHE:
        _NC_CACHE["nc"] = _build_nc()
    nc = _NC_CACHE["nc"]
    consts = _make_consts(min_, scale_)
    in_maps = [
        {"xs": x_flat[c * ROWS_PER_CORE:(c + 1) * ROWS_PER_CORE], "consts": consts}
        for c in range(N_CORES)
    ]
    return run_bass_kernel_spmd(nc, in_maps, list(range(N_CORES)), trace=trace)


def _chunk_last_rows():
    """Global indices g of rows that END an R_t partition chunk (boundary
    pairs (g, g+1) are computed on the host). Excludes the final row."""
    gs = []
    for c in range(N_CORES):
        base = c * ROWS_PER_CORE
        off = 0
        for r in R_LIST:
            p = np.arange(P)
            gs.append(base + off + (p + 1) * r - 1)
            off += P * r
    g = np.concatenate(gs)
    return np.sort(g)[:-1]


def kernel(x, min_, scale_, _trace=False, _return_bkr=False):
    x = np.asarray(x, dtype=np.float32)
    min_ = np.asarray(min_, dtype=np.float32)
    scale_ = np.asarray(scale_, dtype=np.float32)
    x_flat = np.ascontiguousarray(x.reshape(-1, D))

    bkr = _run_device(x_flat, min_, scale_, trace=_trace)
    results = bkr.results

    T = float(ROWS_PER_CORE)
    dev = 0.0
    act = 0.0
    numact = 0.0
    trans = 0.0
    for c in range(N_CORES):
        res = results[c]
        aS = res["accS"].astype(np.float64).reshape(P, -1, 3).sum(axis=(0, 1))
        S_a3lo, S_a3hi, cnt_ne22 = aS
        aD = res["accD"].astype(np.float64).reshape(P, -1, 3).sum(axis=(0, 1))
        cnt_dev, s1, s2 = aD
        act += (T - S_a3lo) / 2.0 + (T + S_a3hi) / 2.0
        dev += cnt_dev
        numact += cnt_ne22
        trans += s1 - s2

    # host-side boundary pairs spanning partition chunks
    g = _chunk_last_rows()
    x3 = x_flat[:, 3]
    m3, s3 = min_[3], scale_[3]
    pa = ((x3[g] - m3) / s3).astype(np.float32)
    ca = ((x3[g + 1] - m3) / s3).astype(np.float32)
    cond = (np.mod(pa, np.float32(2.0)) == 0.0) & (pa < 20.0)
    invalid = (ca != pa + np.float32(1.0)) & (ca != np.float32(22.0))
    trans += np.where(cond, invalid.astype(np.float64), 0.0).sum()

    # Reproduce the reference's f32 summation order exactly.
    t1 = np.float32(dev)
    t2 = np.float32(act)
    t3 = np.float32(trans)
    t4 = np.float32(abs(numact - 58.0))
    out = np.array(((t1 + t2) + t3) + t4, dtype=np.float32)
    if _return_bkr:
        return out, bkr
    return out



# revision 2
# speedup vs baseline: 1.1446x; 1.1446x over previous
"""Trainium2 Bass kernel for nn_CustomPenaltyLayer (MinMax-inverse penalty loss).

Contract: kernel(**inputs) takes the FULL inputs (x:(1024,4096,8) f32,
min_:(8,), scale_:(8,)) and returns the FULL output (scalar f32), sharding
x row-wise across 8 NeuronCores internally.

Math (reference):
  x_inv = (x.reshape(-1, 8) - min_) / scale_
  d = x_inv[:, 2]; a = x_inv[:, 3]
  dev_pen   = count(~(0 <= d <= 252))
  act_pen   = count(a < 0 or a > 22)
  trans_pen = sum over adjacent pairs of [mod(prev,2)==0 & prev<20] *
              [(cur != prev+1) & (cur != 22)]
  num_act   = count(a != 22);  total = dev+act+trans + |num_act - 58|

Device strategy (per core, data-parallel rows). The kernel is DMA-bound
(~47 us to stream 16 MiB/core at ~360 GB/s; only full-row loads are
viable - an 8B-strided column load is descriptor-bound at 7 ns/row).
Compute is restructured so both ACT and DVE sit well under the DMA time:

  - ACT: t3 = Abs(x3*rs3 + (b3-11)) = |a3-11| and t2 = |a2-126| (fused
    deinterleave+affine+abs, one strided pass per column), plus
    Sign(t3-11) with accum_out: act_pen = (N + sum sign)/2 since
    |a3-11| > 11 <=> a3 outside [0,22] (boundary hits are trigger rows).
  - DVE: dev_pen = accum count(t2 > 126) directly (is_gt), and a 3-op
    "t3 is an odd integer" detector via the 2^23 magic-round trick:
    h = t3*0.5 + (M+0.5); r = (h-M)*2; meq = (r-1 == t3). a3 is an even
    integer (the only rows where the transition term or a3==22 can be
    nonzero) iff t3 = |a3-11| is an odd integer. Per-(partition,tile)
    accum of meq is a trigger: chunks with sum==0 contribute exactly 0
    to trans_pen and 0 to count(a3==22).
  - Host: the few triggered chunks (measure-zero for continuous data)
    are recomputed exactly with the reference's f32 arithmetic; partial
    sums are combined on the host into the final scalar.
"""

import os
import sys

for _p in ("/opt/trn_rl_repo", os.path.expanduser("~/.axon_site/_ro/trn_rl_repo")):
    if os.path.isdir(_p) and _p not in sys.path:
        sys.path.append(_p)

import numpy as np

import concourse.bacc as bacc
import concourse.tile as tile
from concourse import mybir
from concourse.bass_utils import run_bass_kernel_spmd

F32 = mybir.dt.float32
ALU = mybir.AluOpType
ACTF = mybir.ActivationFunctionType

MAGIC = 8388608.0  # 2^23
BATCH, TIMESTEPS, D = 1024, 4096, 8
N_ROWS = BATCH * TIMESTEPS          # 4,194,304
N_CORES = 8
ROWS_PER_CORE = N_ROWS // N_CORES   # 524,288
P = 128                             # SBUF partitions
R_LIST = (256, 768, 1024, 1024, 512, 384, 128)   # rows/partition per tile
assert sum(R_LIST) * P == ROWS_PER_CORE
N_T = len(R_LIST)

_NC_CACHE = {}


def _build_nc(x_bufs: int = 3, work_bufs: int = 2):
    n_t = N_T
    nc = bacc.Bacc("TRN2", target_bir_lowering=False, debug=False)

    xs = nc.dram_tensor("xs", [ROWS_PER_CORE, 8], F32, kind="ExternalInput")
    consts = nc.dram_tensor("consts", [P, 8], F32, kind="ExternalInput")
    accA_d = nc.dram_tensor("accA", [P, n_t], F32, kind="ExternalOutput")
    accV_d = nc.dram_tensor("accV", [P, 2 * n_t], F32, kind="ExternalOutput")

    xs_flat = xs.ap()

    with tile.TileContext(nc) as tc:
        with (
            tc.tile_pool(name="xp", bufs=x_bufs) as xp,
            tc.tile_pool(name="wp", bufs=work_bufs) as wp,
            tc.tile_pool(name="acc", bufs=1) as accp,
        ):
            consts_sb = accp.tile([P, 8], F32, tag="consts")
            nc.sync.dma_start(consts_sb[:], consts.ap())
            # Absorb the consts-DMA wait into one dummy ACT op: the HW
            # Activation encoding has a single sync-wait slot, and the
            # loop's first ACT op must wait on the x-tile DMA instead.
            dummy = accp.tile([P, 1], F32, tag="dummy")
            nc.scalar.copy(dummy[:], consts_sb[:, 0:1])
            rs3 = consts_sb[:, 0:1]    # f32(1/scale3)
            b3m11 = consts_sb[:, 1:2]  # -min3*rs3 - 11
            rs2 = consts_sb[:, 2:3]    # f32(1/scale2)
            b2m126 = consts_sb[:, 3:4]  # -min2*rs2 - 126
            n11 = consts_sb[:, 4:5]    # -11.0

            accA = accp.tile([P, n_t], F32, tag="accA")      # ScalarE-owned
            accV = accp.tile([P, 2 * n_t], F32, tag="accV")  # VectorE-owned

            off = 0
            for t, r in enumerate(R_LIST):
                x_t = xp.tile([P, r, 8], F32, tag="x")
                src = xs_flat[off:off + P * r, :].rearrange(
                    "(p r) d -> p r d", r=r)
                nc.sync.dma_start(x_t[:], src)
                off += P * r
                v2 = x_t[:, :, 2]
                v3 = x_t[:, :, 3]

                # ScalarE: fused deinterleave+affine+abs per column, and
                # the act-range sign count: sum sign(t3 - 11) via accum.
                t3 = wp.tile([P, r], F32, tag="t3")
                nc.scalar.activation(t3[:], v3, ACTF.Abs, bias=b3m11, scale=rs3)
                t2 = wp.tile([P, r], F32, tag="t2")
                nc.scalar.activation(t2[:], v2, ACTF.Abs, bias=b2m126, scale=rs2)
                junkA = wp.tile([P, r], F32, tag="junkA")
                nc.scalar.activation(junkA[:], t3[:], ACTF.Sign, bias=n11,
                                     accum_out=accA[:, t:t + 1])

                # VectorE: dev count(t2 > 126) and the odd-integer-t3
                # trigger via the magic-round trick.
                junkV = wp.tile([P, r], F32, tag="junkV")
                nc.vector.tensor_scalar(junkV[:], t2[:], 126.0, 1.0,
                                        ALU.is_gt, ALU.mult,
                                        accum_out=accV[:, 2 * t + 1:2 * t + 2])
                h2 = wp.tile([P, r], F32, tag="h2")
                nc.vector.tensor_scalar(h2[:], t3[:], 0.5, MAGIC + 0.5,
                                        ALU.mult, ALU.add)
                r2p = wp.tile([P, r], F32, tag="r2p")
                nc.vector.tensor_scalar(r2p[:], h2[:], MAGIC, 2.0,
                                        ALU.subtract, ALU.mult)
                junkV2 = wp.tile([P, r], F32, tag="junkV2")
                nc.vector.scalar_tensor_tensor(junkV2[:], r2p[:], 1.0, t3[:],
                                               ALU.subtract, ALU.is_equal,
                                               accum_out=accV[:, 2 * t:2 * t + 1])

            nc.sync.dma_start(accA_d.ap(), accA[:])
            nc.sync.dma_start(accV_d.ap(), accV[:])

    nc.compile()
    return nc


def _make_consts(min_, scale_):
    m = np.asarray(min_, dtype=np.float64)
    s = np.asarray(scale_, dtype=np.float64)
    rs3 = np.float32(1.0) / np.float32(s[3])
    rs2 = np.float32(1.0) / np.float32(s[2])
    b3 = -np.float64(np.float32(m[3])) * np.float64(rs3)
    b2 = -np.float64(np.float32(m[2])) * np.float64(rs2)
    vals = np.array([
        np.float64(rs3),
        b3 - 11.0,
        np.float64(rs2),
        b2 - 126.0,
        -11.0,
        0.0,
        0.0,
        0.0,
    ], dtype=np.float64).astype(np.float32)
    return np.broadcast_to(vals, (P, 8)).copy()


def _run_device(x_flat, min_, scale_, trace=False):
    if "nc" not in _NC_CACHE:
        _NC_CACHE["nc"] = _build_nc()
    nc = _NC_CACHE["nc"]
    consts = _make_consts(min_, scale_)
    in_maps = [
        {"xs": x_flat[c * ROWS_PER_CORE:(c + 1) * ROWS_PER_CORE], "consts": consts}
        for c in range(N_CORES)
    ]
    return run_bass_kernel_spmd(nc, in_maps, list(range(N_CORES)), trace=trace)


def _tile_offsets():
    offs = []
    off = 0
    for r in R_LIST:
        offs.append(off)
        off += P * r
    return offs


def kernel(x, min_, scale_, _trace=False, _return_bkr=False):
    x = np.asarray(x, dtype=np.float32)
    min_ = np.asarray(min_, dtype=np.float32)
    scale_ = np.asarray(scale_, dtype=np.float32)
    x_flat = np.ascontiguousarray(x.reshape(-1, D))

    bkr = _run_device(x_flat, min_, scale_, trace=_trace)
    results = bkr.results

    offs = _tile_offsets()
    act = 0.0
    dev = 0.0
    trans = 0.0
    cnt22 = 0.0
    N_total = float(N_ROWS)

    x3 = x_flat[:, 3]
    m3, s3 = min_[3], scale_[3]

    for c in range(N_CORES):
        res = results[c]
        aA = res["accA"].astype(np.float64)           # (P, n_t): sum sign(t3-11)
        aV = res["accV"].astype(np.float64).reshape(P, N_T, 2)
        meq = aV[:, :, 0]                             # (P, n_t)
        dev += aV[:, :, 1].sum()

        # act from untriggered chunks via the sign identity; triggered
        # chunks (and trans/cnt22) are recomputed exactly on the host.
        trig_p, trig_t = np.nonzero(meq > 0.5)
        r_arr = np.array(R_LIST, dtype=np.float64)
        act_chunks = (r_arr[None, :] + aA) / 2.0
        act += act_chunks.sum()
        for p, t in zip(trig_p, trig_t):
            r = R_LIST[t]
            base = c * ROWS_PER_CORE + offs[t] + p * r
            # exact reference f32 arithmetic on this chunk's rows
            rows = slice(base, base + r)
            a3r = ((x3[rows] - m3) / s3).astype(np.float32)
            act_exact = float(np.sum((a3r < 0.0) | (a3r > np.float32(22.0))))
            act += act_exact - act_chunks[p, t]
            cnt22 += float(np.sum(a3r == np.float32(22.0)))
            # transition pairs whose prev-row lies in this chunk
            hi = min(base + r + 1, N_ROWS)
            a3p = ((x3[base:hi] - m3) / s3).astype(np.float32)
            prev = a3p[:-1]
            cur = a3p[1:]
            cond = (np.mod(prev, np.float32(2.0)) == 0.0) & (prev < 20.0)
            invalid = (cur != prev + np.float32(1.0)) & (cur != np.float32(22.0))
            trans += float(np.where(cond, invalid.astype(np.float64), 0.0).sum())

    numact = N_total - cnt22

    # Reproduce the reference's f32 summation order exactly.
    t1 = np.float32(dev)
    t2 = np.float32(act)
    t3 = np.float32(trans)
    t4 = np.float32(abs(numact - 58.0))
    out = np.array(((t1 + t2) + t3) + t4, dtype=np.float32)
    if _return_bkr:
        return out, bkr
    return out
